# revision 6
# baseline (speedup 1.0000x reference)
"""Trainium2 Bass kernel for nn_CorrelationLayer (441-displacement cost volume).

result[k, i, j] = sum_c f1[c, i, j] * pad(f2)[c, i + dy_k, j + dx_k]
with (dy, dx) in {0, 2, ..., 40}^2, H, W = 48, 64, C = 128, pad D = 20.

Strategy
--------
The contraction over c = 128 maps onto the TensorEngine partition axis.
Each core takes 6 f2 rows of one parity (cores 0-3 even rows, cores 4-7
odd rows); the f1 operand is the 24 same-parity rows.

Per j-group of 5 f1 columns (13 groups, starts 0,5,...,55,59), the
stationary operand is the f1 block [c=128, (j_local, s) = 5*24 = 120]
and the moving operand is the zero-padded f2 block
[c=128, (jp window, r) = 45*6 = 270] stored x-major so each group's
window is a contiguous slice.  One matmul per group produces
M[(jl,s), (xw, r)] = sum_c f1[c, p+2s, jg+jl] * f2p[c, base+2r, jg+xw-20]
covering every (dy, dx) entry; zero padding lands exactly where the
reference's padded correlation is zero.  Host unshard is a pure gather.

Schedule: inputs arrive in 5 chunks split across both HWDGE queues in
the order the matmul stream consumes them; PE warm-up matmuls bridge
the HAM clock-gate window; PSUM is allocated as [120, 2, 512] pair
tiles so two group outputs share one PSUM->SBUF bf16 cast (vector and
scalar alternate); output DMAs are issued per 4-group batch with a
small final DMA to shorten the tail.
"""

import sys
import types

for _p in ("/opt/trn_rl_repo", "/root/.axon_site"):
    if _p not in sys.path:
        sys.path.insert(0, _p)

import ml_dtypes
import numpy as np

BF16 = ml_dtypes.bfloat16

import concourse.bacc as bacc
import concourse.mybir as mybir
from concourse import tile
from concourse import bass_utils
from concourse.bass_utils import run_bass_kernel_spmd

C = 128
H = 48
W = 64
D = 20
ND = 21          # displacements per axis
NCORES = 8
R_ROWS = 6       # f2 rows per core
S_ROWS = 24      # same-parity f1 rows per core
GW = 5           # f1 j-columns per group
WIN = GW + 2 * D  # 45: x window per group
NGRP = 13
JG = [5 * g for g in range(12)] + [59]   # group start j's
MSTAT = GW * S_ROWS   # 120 stationary columns
NMOV = R_ROWS * WIN   # 270 moving columns
F2PX = 2 * D + W      # 104 padded x positions

# combined input layout (columns of the single DRAM param `inp`):
#   [ f1 g0 (120) | f2p x[0:65) (390) | f1 g1-4 (480) | f1 g5-8 (480)
#   | f2p x[25:104) (474) | f1 g9-12 (456) ]
# chunk boundaries; f2p x[25:65) is duplicated so every group's window
# is contiguous inside one chunk.
CH_A = (0, 510)        # f1 g0 + f2p x[0:65)
CH_B = (510, 990)      # f1 g1-4
CH_C = (990, 1470)     # f1 g5-8
CH_D = (1470, 1944)    # f2p x[25:104)
CH_E = (1944, 2400)    # f1 g9-12
INP_COLS = 2400


def _ensure_ntff_hook():
    """Register the axon NTFF profile hook if possible (for trace runs)."""
    try:
        import antenv
        if "antenv.axon_hooks" not in sys.modules:
            mod = types.ModuleType("antenv.axon_hooks")
            _h = [None]
            mod.set_axon_ntff_profile_hook = lambda h: _h.__setitem__(0, h)
            mod.get_axon_ntff_profile_hook = lambda: _h[0]
            sys.modules["antenv.axon_hooks"] = mod
            antenv.axon_hooks = mod
        bass_utils.upload_artifacts = lambda tmpdir: "local://" + tmpdir
        from trn_agent_boot.trn_boot import _ntff_profile_via_ctypes
        sys.modules["antenv.axon_hooks"].set_axon_ntff_profile_hook(
            _ntff_profile_via_ctypes("/opt/axon/libaxon_pjrt.so")
        )
    except Exception:
        pass


def build_program():
    nc = bacc.Bacc(None, target_bir_lowering=False)
    inp = nc.declare_dram_parameter("inp", [C, INP_COLS], mybir.dt.bfloat16, isOutput=False)
    mout = nc.declare_dram_parameter(
        "mout", [MSTAT, NGRP * NMOV], mybir.dt.bfloat16, isOutput=True
    )

    with tile.TileContext(nc) as tc:
        with (
            tc.tile_pool(name="in", bufs=1) as in_pool,
            tc.tile_pool(name="out", bufs=1) as out_pool,
            tc.tile_pool(name="ps", bufs=3, space="PSUM") as ps_pool,
            tc.tile_pool(name="ps1", bufs=1, space="PSUM") as ps1_pool,
            tc.tile_pool(name="pswarm", bufs=1, space="PSUM") as psw_pool,
        ):
            # input chunks in stream-consumption order; sync queue takes the
            # first three, scalar (whose queue head holds the ACT table
            # load) takes the later two.
            chunks = [CH_A, CH_B, CH_C, CH_D, CH_E]
            qengs = [nc.sync, nc.sync, nc.sync, nc.scalar, nc.scalar]
            t = []
            for q, ((a, b), eng) in enumerate(zip(chunks, qengs)):
                tl = in_pool.tile([C, b - a], mybir.dt.bfloat16, tag=f"in{q}")
                eng.dma_start(out=tl[:], in_=inp[:, a:b])
                t.append(tl)
            t_A, t_B, t_C, t_D, t_E = t

            # PE warm-up on a vector-memset scratch tile: keeps the PE busy
            # from ~window start so the HAM clock gate reaches 2.4 GHz
            # before the real matmuls.
            scratch = in_pool.tile([C, 512], mybir.dt.bfloat16, tag="scratch")
            nc.vector.memset(scratch[:], 0)
            ps_warm = psw_pool.tile([128, 512], mybir.dt.float32, tag="psw")
            for _ in range(5):
                nc.tensor.matmul(
                    ps_warm[:], scratch[:, :128], scratch[:], start=True, stop=True
                )

            def lhsT_ap(g):
                jg = JG[g]
                if g == 0:
                    return t_A[:, 0:MSTAT]
                if g <= 4:
                    return t_B[:, (jg - 5) * S_ROWS : (jg - 5) * S_ROWS + MSTAT]
                if g <= 8:
                    return t_C[:, (jg - 25) * S_ROWS : (jg - 25) * S_ROWS + MSTAT]
                return t_E[:, (jg - 45) * S_ROWS : (jg - 45) * S_ROWS + MSTAT]

            def rhs_ap(g):
                jg = JG[g]
                if g <= 4:
                    # t_A holds f2p x[0:65) at column offset 120
                    return t_A[:, 120 + jg * R_ROWS : 120 + (jg + WIN) * R_ROWS]
                # t_D holds f2p x[25:104)
                return t_D[:, (jg - 25) * R_ROWS : (jg + 20) * R_ROWS]

            out_sb = out_pool.tile([MSTAT, NGRP * NMOV], mybir.dt.bfloat16)
            # pair psum tiles: two groups per tile (one bank each half);
            # one cast moves both groups' output (fewer fixed overheads)
            for k in range(NGRP // 2):
                ps = ps_pool.tile([MSTAT, 2, 512], mybir.dt.float32, tag="ps")
                for half in range(2):
                    g = 2 * k + half
                    nc.tensor.matmul(
                        ps[:, half, 0:NMOV], lhsT_ap(g), rhs_ap(g), start=True, stop=True
                    )
                dst = out_sb[:, 2 * k * NMOV : (2 * k + 2) * NMOV]
                if k % 2 == 0:
                    nc.vector.tensor_copy(dst, ps[:, :, 0:NMOV])
                else:
                    nc.scalar.copy(dst, ps[:, :, 0:NMOV])
                if k == 1:   # groups 0-3 cast
                    nc.sync.dma_start(out=mout[:, 0 : 4 * NMOV], in_=out_sb[:, 0 : 4 * NMOV])
                elif k == 3:  # groups 4-7 cast
                    nc.sync.dma_start(
                        out=mout[:, 4 * NMOV : 8 * NMOV], in_=out_sb[:, 4 * NMOV : 8 * NMOV]
                    )
            # final single group 12
            g = NGRP - 1
            ps_last = ps1_pool.tile([MSTAT, NMOV], mybir.dt.float32, tag="pslast")
            nc.tensor.matmul(ps_last[:], lhsT_ap(g), rhs_ap(g), start=True, stop=True)
            nc.vector.tensor_copy(out_sb[:, g * NMOV : (g + 1) * NMOV], ps_last[:])
            # groups 8-11 (casts done after pair k=5 on scalar)
            nc.scalar.dma_start(
                out=mout[:, 8 * NMOV : 12 * NMOV], in_=out_sb[:, 8 * NMOV : 12 * NMOV]
            )
            # small final DMA: group 12 only
            nc.sync.dma_start(
                out=mout[:, 12 * NMOV : 13 * NMOV], in_=out_sb[:, 12 * NMOV : 13 * NMOV]
            )
    nc.compile()
    return nc


_PROGRAM_CACHE = {}


def _get_program():
    if "nc" not in _PROGRAM_CACHE:
        _PROGRAM_CACHE["nc"] = build_program()
    return _PROGRAM_CACHE["nc"]


def _shard_inputs(features_1, features_2):
    """Per-core input maps. Core m < 4: even f2 rows 12m..12m+10; core m >= 4:
    odd rows 12(m-4)+1..12(m-4)+11. f1 operand = the 24 same-parity rows,
    j-major (col = j*24 + s); f2 rows zero-padded in x by D=20, x-major
    (col = x*6 + r). Combined into one arrival-ordered input tensor."""
    f1 = np.ascontiguousarray(features_1, dtype=np.float32)
    f2 = np.ascontiguousarray(features_2, dtype=np.float32)
    in_maps = []
    for m in range(NCORES):
        p = 0 if m < 4 else 1
        base = 12 * m if m < 4 else 12 * (m - 4) + 1
        f1p = f1[:, p::2, :]                                   # [C, 24, 64]
        f1j = np.ascontiguousarray(f1p.transpose(0, 2, 1)).reshape(C, W * S_ROWS)
        rows = base + 2 * np.arange(R_ROWS)
        f2p = np.zeros((C, F2PX, R_ROWS), dtype=np.float32)    # x-major
        f2p[:, D : D + W, :] = f2[:, rows, :].transpose(0, 2, 1)
        f2px = f2p.reshape(C, F2PX * R_ROWS)
        inp = np.concatenate(
            [
                f1j[:, 0:120],               # g0 stationary
                f2px[:, 0 : 65 * R_ROWS],    # x[0:65)
                f1j[:, 120:600],             # g1-4
                f1j[:, 600:1080],            # g5-8
                f2px[:, 25 * R_ROWS :],      # x[25:104)
                f1j[:, 1080:1536],           # g9-12
            ],
            axis=1,
        )
        in_maps.append({"inp": inp.astype(BF16)})
    return in_maps


def _assemble(results):
    """Gather out[dy, dx, i, j] from the per-core group matmul tiles."""
    Mall = np.empty((NCORES, NGRP, MSTAT, NMOV), dtype=np.float32)
    for m in range(NCORES):
        raw = np.asarray(results[m]["mout"]).astype(np.float32)
        Mall[m] = raw.reshape(MSTAT, NGRP, NMOV).transpose(1, 0, 2)

    dy, dxi, i, j = np.ogrid[0:ND, 0:ND, 0:H, 0:W]
    r2 = i + 2 * dy - 20
    valid = (r2 >= 0) & (r2 < H)
    r2c = np.clip(r2, 0, H - 1)
    par = r2c & 1
    r2h = r2c >> 1
    core = par * 4 + r2h // R_ROWS
    r = r2h % R_ROWS
    s = (i - par) // 2
    g = np.where(j < 60, j // GW, NGRP - 1)
    jl = np.where(j < 60, j % GW, j - JG[-1])
    m_idx = jl * S_ROWS + s
    n_idx = (jl + 2 * dxi) * R_ROWS + r     # x-major f2p: col = xw*6 + r
    out = np.where(valid, Mall[core, g, m_idx, n_idx], np.float32(0.0))
    return out.reshape(1, ND * ND, H, W)


def kernel(features_1, features_2):
    nc = _get_program()
    in_maps = _shard_inputs(features_1, features_2)
    res = run_bass_kernel_spmd(nc, in_maps, list(range(NCORES)))
    return _assemble(res.results)


def kernel_traced(features_1, features_2, tmpdir=None):
    """Same as kernel() but with NTFF profiling; returns (output, exec_time_ns)."""
    _ensure_ntff_hook()
    nc = _get_program()
    in_maps = _shard_inputs(features_1, features_2)
    res = run_bass_kernel_spmd(
        nc, in_maps, list(range(NCORES)), trace=True, tmpdir=tmpdir
    )
    return _assemble(res.results), res.exec_time_ns


# revision 8
# speedup vs baseline: 1.1494x; 1.1494x over previous
"""Trainium2 Bass kernel for nn_CorrelationLayer (441-displacement cost volume).

result[k, i, j] = sum_c f1[c, i, j] * pad(f2)[c, i + dy_k, j + dx_k]
with (dy, dx) in {0, 2, ..., 40}^2, H, W = 48, 64, C = 128, pad D = 20.

Strategy
--------
The contraction over c = 128 maps onto the TensorEngine partition axis.
Each core takes 6 f2 rows of one parity (cores 0-3 even rows, cores 4-7
odd rows); the f1 operand is the 24 same-parity rows.

Per j-group of 5 f1 columns (13 groups, starts 0,5,...,55,59), the
stationary operand is the f1 block [c=128, (j_local, s) = 5*24 = 120]
and the moving operand is the zero-padded f2 block stored x-major
([c, (x window, r)]), trimmed to the valid (non-pad) x range per group
so no all-zero columns are computed, cast, or written.  One matmul per
group produces
M[(jl,s), (xw, r)] = sum_c f1[c, p+2s, jg+jl] * f2p[c, base+2r, jg+xw-20];
out-of-range displacements are zeros the host fills in.  Host unshard
is a pure gather.

Schedule: inputs arrive as 3 chunks on the sync HWDGE queue in strict
consumption order (per-queue FIFO guarantees arrival order); PE warm-up
matmuls bridge the HAM clock-gate window; PSUM is allocated as
[120, 2, 512] pair tiles so two group outputs share one PSUM->SBUF
bf16 cast (vector and scalar split the pairs greedily); output DMAs
are issued per cast-pair batch with a small final DMA for the tail.
"""

import sys
import types

for _p in ("/opt/trn_rl_repo", "/root/.axon_site"):
    if _p not in sys.path:
        sys.path.insert(0, _p)

import ml_dtypes
import numpy as np

BF16 = ml_dtypes.bfloat16

import concourse.bacc as bacc
import concourse.mybir as mybir
from concourse import tile
from concourse import bass_utils
from concourse.bass_utils import run_bass_kernel_spmd

C = 128
H = 48
W = 64
D = 20
ND = 21          # displacements per axis
NCORES = 8
R_ROWS = 6       # f2 rows per core
S_ROWS = 24      # same-parity f1 rows per core
GW = 5           # f1 j-columns per group
NGRP = 13
JG = [5 * g for g in range(12)] + [59]   # group start j's
MSTAT = GW * S_ROWS   # 120 stationary columns

# trimmed x-window per group: padded x in [lo, hi) with
# lo = max(jg, 20), hi = min(jg + 45, 84); pad cols outside are zero.
XLO = [max(jg, D) for jg in JG]
XHI = [min(jg + 45, D + W) for jg in JG]
WG = [hi - lo for lo, hi in zip(XLO, XHI)]        # 25..45
NG = [R_ROWS * w for w in WG]                      # moving cols per group

# f2p slices staged in SBUF (x-major, col = x*6 + r):
#   piece 1 (in chunk A): x[20:65)  -> groups 0-4
#   piece 2 (in chunk D): x[25:84)  -> groups 5-12
F2A_X0, F2A_X1 = 20, 65
F2D_X0, F2D_X1 = 25, 84

# combined input layout (columns of the single DRAM param `inp`):
LEN_F1G0 = MSTAT
LEN_F2A = (F2A_X1 - F2A_X0) * R_ROWS   # 270
LEN_F1B = 4 * GW * S_ROWS              # 480, g1-4
LEN_F1C = 4 * GW * S_ROWS              # 480, g5-8
LEN_F2D = (F2D_X1 - F2D_X0) * R_ROWS   # 354
LEN_F1E = 1536 - 1080                  # 456, g9-12
O_F1G0 = 0
O_F2A = O_F1G0 + LEN_F1G0              # 120
O_F1B = O_F2A + LEN_F2A                # 390
O_F1C = O_F1B + LEN_F1B                # 870
O_F2D = O_F1C + LEN_F1C                # 1350
O_F1E = O_F2D + LEN_F2D                # 1704
INP_COLS = O_F1E + LEN_F1E             # 2160
# chunk boundaries (3 DMAs, strict consumption order on one queue)
CH1 = (0, O_F1C)          # f1 g0 + f2pA + f1 g1-4
CH2 = (O_F1C, O_F1E)      # f1 g5-8 + f2pD
CH3 = (O_F1E, INP_COLS)   # f1 g9-12

# output packing: pairs (2k, 2k+1) share a cast with width 2*maxN (the
# narrower group's tail inside the pair is junk the host skips); the
# final group 12 is a single.
PAIR_W = [2 * max(NG[2 * k], NG[2 * k + 1]) for k in range(NGRP // 2)]
PAIR_OFF = [0]
for w in PAIR_W:
    PAIR_OFF.append(PAIR_OFF[-1] + w)
OUT_COLS = PAIR_OFF[-1] + NG[-1]
GOFF = []
for g in range(NGRP - 1):
    GOFF.append(PAIR_OFF[g // 2] + (PAIR_W[g // 2] // 2 if g % 2 else 0))
GOFF.append(PAIR_OFF[-1])


def _ensure_ntff_hook():
    """Register the axon NTFF profile hook if possible (for trace runs)."""
    try:
        import antenv
        if "antenv.axon_hooks" not in sys.modules:
            mod = types.ModuleType("antenv.axon_hooks")
            _h = [None]
            mod.set_axon_ntff_profile_hook = lambda h: _h.__setitem__(0, h)
            mod.get_axon_ntff_profile_hook = lambda: _h[0]
            sys.modules["antenv.axon_hooks"] = mod
            antenv.axon_hooks = mod
        bass_utils.upload_artifacts = lambda tmpdir: "local://" + tmpdir
        from trn_agent_boot.trn_boot import _ntff_profile_via_ctypes
        sys.modules["antenv.axon_hooks"].set_axon_ntff_profile_hook(
            _ntff_profile_via_ctypes("/opt/axon/libaxon_pjrt.so")
        )
    except Exception:
        pass


def build_program():
    nc = bacc.Bacc(None, target_bir_lowering=False)
    inp = nc.declare_dram_parameter("inp", [C, INP_COLS], mybir.dt.bfloat16, isOutput=False)
    mout = nc.declare_dram_parameter(
        "mout", [MSTAT, OUT_COLS], mybir.dt.bfloat16, isOutput=True
    )

    with tile.TileContext(nc) as tc:
        with (
            tc.tile_pool(name="in", bufs=1) as in_pool,
            tc.tile_pool(name="out", bufs=1) as out_pool,
            tc.tile_pool(name="ps", bufs=3, space="PSUM") as ps_pool,
            tc.tile_pool(name="ps1", bufs=1, space="PSUM") as ps1_pool,
            tc.tile_pool(name="pswarm", bufs=1, space="PSUM") as psw_pool,
        ):
            # input chunks: one queue, strict consumption order (per-queue
            # FIFO makes transfers complete in this order)
            t = []
            for q, (a, b) in enumerate([CH1, CH2, CH3]):
                tl = in_pool.tile([C, b - a], mybir.dt.bfloat16, tag=f"in{q}")
                nc.sync.dma_start(out=tl[:], in_=inp[:, a:b])
                t.append(tl)
            t_1, t_2, t_3 = t

            # PE warm-up on a vector-memset scratch tile: keeps the PE busy
            # from ~window start so the HAM clock gate reaches 2.4 GHz
            # before the real matmuls.
            scratch = in_pool.tile([C, 512], mybir.dt.bfloat16, tag="scratch")
            nc.vector.memset(scratch[:], 0)
            ps_warm = psw_pool.tile([128, 512], mybir.dt.float32, tag="psw")
            for _ in range(5):
                nc.tensor.matmul(
                    ps_warm[:], scratch[:, :128], scratch[:], start=True, stop=True
                )

            def lhsT_ap(g):
                jg = JG[g]
                if g == 0:
                    return t_1[:, O_F1G0 : O_F1G0 + MSTAT]
                if g <= 4:
                    lo = O_F1B + (jg - 5) * S_ROWS
                    return t_1[:, lo : lo + MSTAT]
                if g <= 8:
                    lo = (jg - 25) * S_ROWS          # t_2 starts at O_F1C
                    return t_2[:, lo : lo + MSTAT]
                lo = (jg - 45) * S_ROWS              # t_3 starts at O_F1E
                return t_3[:, lo : lo + MSTAT]

            def rhs_ap(g):
                if g <= 4:
                    lo = O_F2A + (XLO[g] - F2A_X0) * R_ROWS
                    return t_1[:, lo : lo + NG[g]]
                lo = (O_F2D - O_F1C) + (XLO[g] - F2D_X0) * R_ROWS
                return t_2[:, lo : lo + NG[g]]

            out_sb = out_pool.tile([MSTAT, OUT_COLS], mybir.dt.bfloat16)
            # pair psum tiles: two groups per tile (one bank per half);
            # one cast moves both groups' output.  greedy engine split:
            # vector takes p0, p2, p4 + the final single; scalar p1, p3, p5.
            for k in range(NGRP // 2):
                ps = ps_pool.tile([MSTAT, 2, 512], mybir.dt.float32, tag="ps")
                half_n = PAIR_W[k] // 2
                for half in range(2):
                    g = 2 * k + half
                    nc.tensor.matmul(
                        ps[:, half, 0 : NG[g]], lhsT_ap(g), rhs_ap(g),
                        start=True, stop=True,
                    )
                dst = out_sb[:, PAIR_OFF[k] : PAIR_OFF[k + 1]]
                if k % 2 == 0:
                    nc.vector.tensor_copy(dst, ps[:, :, 0:half_n])
                else:
                    nc.scalar.copy(dst, ps[:, :, 0:half_n])
                if k == 1:   # pairs 0-1 (groups 0-3) cast
                    nc.sync.dma_start(
                        out=mout[:, 0 : PAIR_OFF[2]], in_=out_sb[:, 0 : PAIR_OFF[2]]
                    )
                elif k == 3:  # pairs 2-3 (groups 4-7) cast
                    nc.sync.dma_start(
                        out=mout[:, PAIR_OFF[2] : PAIR_OFF[4]],
                        in_=out_sb[:, PAIR_OFF[2] : PAIR_OFF[4]],
                    )
                elif k == 5:  # pairs 4-5 (groups 8-11) cast
                    nc.sync.dma_start(
                        out=mout[:, PAIR_OFF[4] : PAIR_OFF[6]],
                        in_=out_sb[:, PAIR_OFF[4] : PAIR_OFF[6]],
                    )
            # final single group 12
            g = NGRP - 1
            ps_last = ps1_pool.tile([MSTAT, 512], mybir.dt.float32, tag="pslast")
            nc.tensor.matmul(ps_last[:, 0 : NG[g]], lhsT_ap(g), rhs_ap(g),
                             start=True, stop=True)
            nc.vector.tensor_copy(
                out_sb[:, GOFF[g] : GOFF[g] + NG[g]], ps_last[:, 0 : NG[g]]
            )
            # small final DMA on the other queue
            nc.scalar.dma_start(
                out=mout[:, GOFF[g] : GOFF[g] + NG[g]],
                in_=out_sb[:, GOFF[g] : GOFF[g] + NG[g]],
            )
    nc.compile()
    return nc


_PROGRAM_CACHE = {}


def _get_program():
    if "nc" not in _PROGRAM_CACHE:
        _PROGRAM_CACHE["nc"] = build_program()
    return _PROGRAM_CACHE["nc"]


def _shard_inputs(features_1, features_2):
    """Per-core input maps. Core m < 4: even f2 rows 12m..12m+10; core m >= 4:
    odd rows 12(m-4)+1..12(m-4)+11. f1 operand = the 24 same-parity rows,
    j-major (col = j*24 + s); f2 rows zero-padded in x by D=20, x-major
    (col = x*6 + r). Combined into one arrival-ordered input tensor."""
    f1 = np.ascontiguousarray(features_1, dtype=np.float32)
    f2 = np.ascontiguousarray(features_2, dtype=np.float32)
    in_maps = []
    for m in range(NCORES):
        p = 0 if m < 4 else 1
        base = 12 * m if m < 4 else 12 * (m - 4) + 1
        f1p = f1[:, p::2, :]                                   # [C, 24, 64]
        f1j = np.ascontiguousarray(f1p.transpose(0, 2, 1)).reshape(C, W * S_ROWS)
        rows = base + 2 * np.arange(R_ROWS)
        f2p = np.zeros((C, 2 * D + W, R_ROWS), dtype=np.float32)    # x-major
        f2p[:, D : D + W, :] = f2[:, rows, :].transpose(0, 2, 1)
        f2px = f2p.reshape(C, (2 * D + W) * R_ROWS)
        inp = np.concatenate(
            [
                f1j[:, 0:120],                                   # g0 stationary
                f2px[:, F2A_X0 * R_ROWS : F2A_X1 * R_ROWS],      # x[20:65)
                f1j[:, 120:600],                                 # g1-4
                f1j[:, 600:1080],                                # g5-8
                f2px[:, F2D_X0 * R_ROWS : F2D_X1 * R_ROWS],      # x[25:84)
                f1j[:, 1080:1536],                               # g9-12
            ],
            axis=1,
        )
        in_maps.append({"inp": inp.astype(BF16)})
    return in_maps


def _assemble(results):
    """Gather out[dy, dx, i, j] from the per-core packed matmul tiles."""
    Mall = np.empty((NCORES, MSTAT, OUT_COLS), dtype=np.float32)
    for m in range(NCORES):
        Mall[m] = np.asarray(results[m]["mout"]).astype(np.float32)

    goff = np.asarray(GOFF)
    xlo_rel = np.asarray([XLO[g] - JG[g] for g in range(NGRP)])   # trim per group
    wg = np.asarray(WG)

    dy, dxi, i, j = np.ogrid[0:ND, 0:ND, 0:H, 0:W]
    r2 = i + 2 * dy - 20
    valid = (r2 >= 0) & (r2 < H)
    r2c = np.clip(r2, 0, H - 1)
    par = r2c & 1
    r2h = r2c >> 1
    core = par * 4 + r2h // R_ROWS
    r = r2h % R_ROWS
    s = (i - par) // 2
    g = np.where(j < 60, j // GW, NGRP - 1)
    jl = np.where(j < 60, j % GW, j - JG[-1])
    xw = jl + 2 * dxi                       # window coordinate, 0..44
    xrel = xw - xlo_rel[g]                  # trimmed-window coordinate
    validx = (xrel >= 0) & (xrel < wg[g])   # pad region -> zero
    xrelc = np.clip(xrel, 0, None)
    m_idx = jl * S_ROWS + s
    n_idx = goff[g] + xrelc * R_ROWS + r
    n_idx = np.minimum(n_idx, OUT_COLS - 1)
    out = np.where(valid & validx, Mall[core, m_idx, n_idx], np.float32(0.0))
    return out.reshape(1, ND * ND, H, W)


def kernel(features_1, features_2):
    nc = _get_program()
    in_maps = _shard_inputs(features_1, features_2)
    res = run_bass_kernel_spmd(nc, in_maps, list(range(NCORES)))
    return _assemble(res.results)


def kernel_traced(features_1, features_2, tmpdir=None):
    """Same as kernel() but with NTFF profiling; returns (output, exec_time_ns)."""
    _ensure_ntff_hook()
    nc = _get_program()
    in_maps = _shard_inputs(features_1, features_2)
    res = run_bass_kernel_spmd(
        nc, in_maps, list(range(NCORES)), trace=True, tmpdir=tmpdir
    )
    return _assemble(res.results), res.exec_time_ns
